# revision 32
# baseline (speedup 1.0000x reference)
"""GQA attention kernel for Trainium2, 8-core tensor-parallel.

Sharding: core c handles batch b=c//4 and kv-head pair {2*(c%4), 2*(c%4)+1}
(8 q heads). q/k/v projections column-sharded, out_proj row-sharded; the
4 partial out_proj products per batch are summed on host (the gather).

Everything on-device is feature-major ([feat, token]) so every matmul
contraction dim lands on partitions. float32r (tf32-like, full PE rate for
free-dim>=256) is used for all matmuls. Softmax has no max-subtraction
(scores are O(1) here) so exp needs no rescaling pass; the softmax
denominator comes free from a ones-column appended to V.
"""
import sys
if "/opt/trn_rl_repo" not in sys.path:
    sys.path.insert(0, "/opt/trn_rl_repo")
import numpy as np

HID = 2048
L = 2048
D = 64
NCORE = 8
NKT = HID // 128        # 16 k-tiles over hidden
NCH = 4                 # token chunks of 512 for projections
CH = 512
NLT = L // 128          # 16 lk tiles
LQC = 1024              # lq chunk for attention
BIG = -1e32

_cached = {}


def _build():
    import concourse.bass as bass
    from concourse import bacc
    import concourse.mybir as mybir
    import concourse.tile as tile

    F32R = mybir.dt.float32r
    F32 = mybir.dt.float32
    EXP = mybir.ActivationFunctionType.Exp

    nc = bacc.Bacc(None, target_bir_lowering=False)
    xT = nc.dram_tensor("xT", [128, NKT, L], F32R, kind="ExternalInput")
    qw = nc.dram_tensor("qw", [128, NKT, 512], F32R, kind="ExternalInput")
    kw = nc.dram_tensor("kw", [128, NKT, 128], F32R, kind="ExternalInput")
    vw = nc.dram_tensor("vw", [128, NKT, 128], F32R, kind="ExternalInput")
    ow = nc.dram_tensor("ow", [128, 4, HID], F32R, kind="ExternalInput")
    qb = nc.dram_tensor("qb", [128, 4], F32, kind="ExternalInput")
    kb = nc.dram_tensor("kb", [128, 1], F32, kind="ExternalInput")
    vb = nc.dram_tensor("vb", [128, 1], F32, kind="ExternalInput")
    ident = nc.dram_tensor("ident", [128, 128], F32, kind="ExternalInput")
    BF16 = mybir.dt.bfloat16
    triT = nc.dram_tensor("triT", [128, 128], BF16, kind="ExternalInput")
    idnb = nc.dram_tensor("idnb", [128, 128], BF16, kind="ExternalInput")
    outp = nc.dram_tensor("outp", [NKT, 128, L], F32, kind="ExternalOutput")

    with tile.TileContext(nc) as tc:
        with tc.tile_pool(name="cst", bufs=1) as cst, \
             tc.tile_pool(name="res", bufs=1) as res:
            idn = cst.tile([128, 128], F32)
            triT_sb = cst.tile([128, 128], BF16)
            idnb_sb = cst.tile([128, 128], BF16)
            qb_sb = cst.tile([128, 4], F32)
            kb_sb = cst.tile([128, 1], F32)
            vb_sb = cst.tile([128, 1], F32)
            def load_consts():
                for dst, src in [(idn, ident), (triT_sb, triT), (idnb_sb, idnb),
                                 (qb_sb, qb), (kb_sb, kb), (vb_sb, vb)]:
                    nc.sync.dma_start(out=dst, in_=src.ap())

            qT_sb = res.tile([128, 4, L], F32R)   # head h: parts 64*(h//4), tile h%4
            kT_sb = res.tile([128, L], F32R)      # kv j at parts 64j
            v_aug = res.tile([128, NLT, 130], F32R)
            yT_c0 = res.tile([128, 4, LQC], F32R)
            yT_c1 = res.tile([128, 4, LQC], F32R)
            yT_cs = [yT_c0, yT_c1]
            nc.vector.memset(v_aug[:, :, 64:65].bitcast(F32), 1.0)
            nc.vector.memset(v_aug[:, :, 129:130].bitcast(F32), 1.0)

            # ---- Phase A: projections (feature-major) + inline V transpose ----
            with tc.tile_pool(name="wqkv", bufs=1) as wpool, \
                 tc.tile_pool(name="xc", bufs=2) as xcp, \
                 tc.tile_pool(name="vtmp", bufs=2) as vtp, \
                 tc.tile_pool(name="pps", bufs=1, space="PSUM") as pps, \
                 tc.tile_pool(name="trps", bufs=2, space="PSUM") as trps:
                qw_sb = wpool.tile([128, NKT, 512], F32R)
                kw_sb = wpool.tile([128, NKT, 128], F32R)
                vw_sb = wpool.tile([128, NKT, 128], F32R)
                # DMA order = issue order: k/v weights and the first x tiles
                # land before the bulky qw stream so PE can start ~7us in.
                nc.sync.dma_start(out=kw_sb[:, 0:2, :], in_=kw.ap()[:, 0:2, :])
                nc.sync.dma_start(out=vw_sb[:, 0:2, :], in_=vw.ap()[:, 0:2, :])
                nc.sync.dma_start(out=qw_sb[:, 0:2, :], in_=qw.ap()[:, 0:2, :])
                qw_loaded = [False] * NKT
                qw_loaded[0] = qw_loaded[1] = True
                for nt in range(NCH):
                    sl = slice(CH * nt, CH * nt + CH)
                    q_ps = [pps.tile([128, CH], F32, tag=f"qps{m}", name=f"qps{m}") for m in range(4)]
                    k_ps = pps.tile([128, CH], F32, tag="kps")
                    v_ps = pps.tile([128, CH], F32, tag="vps")
                    for half in range(2):
                        xc = []
                        for i in range(8):
                            t = xcp.tile([128, CH], F32R, tag=f"xc{i}")
                            kt = 8 * half + i
                            nc.sync.dma_start(
                                out=t, in_=xT.ap()[:, kt, CH * nt:CH * nt + CH])
                            xc.append(t)
                        if nt == 0 and half == 0:
                            nc.sync.dma_start(out=kw_sb[:, 2:8, :],
                                              in_=kw.ap()[:, 2:8, :])
                            nc.sync.dma_start(out=vw_sb[:, 2:8, :],
                                              in_=vw.ap()[:, 2:8, :])
                            for kt in range(2, 8):
                                nc.sync.dma_start(out=qw_sb[:, kt, :],
                                                  in_=qw.ap()[:, kt, :])
                                qw_loaded[kt] = True
                        if nt == 0 and half == 1:
                            nc.sync.dma_start(out=kw_sb[:, 8:NKT, :],
                                              in_=kw.ap()[:, 8:NKT, :])
                            nc.sync.dma_start(out=vw_sb[:, 8:NKT, :],
                                              in_=vw.ap()[:, 8:NKT, :])
                            load_consts()
                        if not qw_loaded[8 * half]:
                            for kt in range(8 * half, 8 * half + 8):
                                nc.sync.dma_start(out=qw_sb[:, kt, :],
                                                  in_=qw.ap()[:, kt, :])
                                qw_loaded[kt] = True
                        for i in range(8):
                            kt = 8 * half + i
                            st, sp = kt == 0, kt == NKT - 1
                            nc.tensor.matmul(k_ps, kw_sb[:, kt, :], xc[i],
                                             start=st, stop=sp)
                            nc.tensor.matmul(v_ps, vw_sb[:, kt, :], xc[i],
                                             start=st, stop=sp)
                            for mt in range(4):
                                nc.tensor.matmul(
                                    q_ps[mt], qw_sb[:, kt, 128 * mt:128 * mt + 128],
                                    xc[i], start=st, stop=sp)
                    IDN = mybir.ActivationFunctionType.Identity
                    for mt in range(4):
                        if mt % 2 == 0:
                            nc.scalar.activation(out=qT_sb[:, mt, sl], in_=q_ps[mt],
                                                 func=IDN, bias=qb_sb[:, mt:mt + 1])
                        else:
                            nc.vector.tensor_scalar_add(out=qT_sb[:, mt, sl],
                                                        in0=q_ps[mt],
                                                        scalar1=qb_sb[:, mt:mt + 1])
                    nc.vector.tensor_scalar_add(out=kT_sb[:, sl], in0=k_ps,
                                                scalar1=kb_sb)
                    vtmp = vtp.tile([128, CH], F32, tag="vt")
                    nc.scalar.activation(out=vtmp, in_=v_ps, func=IDN, bias=vb_sb)
                    for tt in range(CH // 128):
                        t = (CH * nt) // 128 + tt
                        tr_ps = trps.tile([128, 128], F32, tag="tr")
                        nc.tensor.transpose(tr_ps, vtmp[:, 128 * tt:128 * tt + 128], idn)
                        nc.vector.tensor_copy(out=v_aug[:, t, 0:64], in_=tr_ps[:, 0:64])
                        nc.vector.tensor_copy(out=v_aug[:, t, 65:129], in_=tr_ps[:, 64:128])

            # ---- Phase C: attention, scores transposed [lk, lq] ----
            # ow preloaded here (in 4 chunks) — DMA engines are idle during
            # attention, and Phase D then never waits on weight loads.
            owp = tc.tile_pool(name="owp", bufs=1)
            ow_sb = owp.__enter__().tile([128, 4, HID], F32R)
            for q4 in range(4):
                nc.sync.dma_start(out=ow_sb[:, q4, :], in_=ow.ap()[:, q4, :])
            with tc.tile_pool(name="work", bufs=3) as work:

                def head_tail(c, h, pv_ps):
                    # softmax denominator + y write; recip split on DVE so it
                    # never head-of-line-blocks, broadcast on gpsimd.
                    mt = h % 4
                    recip = work.tile([1, LQC], F32, tag="recip")
                    nc.vector.reciprocal(recip, pv_ps[64:65, :])
                    bcast = work.tile([64, LQC], F32, tag="bcast")
                    nc.gpsimd.partition_broadcast(bcast, recip)
                    if h < 4:
                        nc.vector.tensor_mul(out=yT_cs[c][0:64, mt, :],
                                             in0=pv_ps[0:64, :], in1=bcast)
                    else:
                        ytmp = work.tile([64, LQC], F32R, tag="ytmp")
                        nc.vector.tensor_mul(out=ytmp, in0=pv_ps[0:64, :], in1=bcast)
                        nc.sync.dma_start(out=yT_cs[c][64:128, mt, :], in_=ytmp)

                def emit_head(c, h, scps, pvps, wide):
                    """One head's scores->exp->pv loop. wide=True: [128,LQC]
                    sc tiles; wide=False: [128,512] sc tiles (seg-granular),
                    for the reduced-PSUM final scope."""
                    nonlocal pending_pv
                    base, mt, j = 64 * (h // 4), h % 4, h // 4
                    pv_ps = pvps.tile([65, LQC], F32, tag="pv")
                    ntile = (LQC // 128) * c + (LQC // 128)
                    # ascending t is REQUIRED: tile t=0 spans the full
                    # [0:LQC] column range, so its start=True clears the whole
                    # pv accumulation region; narrower tiles only cover
                    # [o:LQC] and must come after.
                    for ti, t in enumerate(range(ntile)):
                        o = max(0, 128 * t - LQC * c)
                        segs = [(o, 512), (512, LQC)] if o < 512 else [(o, LQC)]
                        if wide:
                            sc_ps = scps.tile([128, LQC], F32, tag="sc")
                            expS = work.tile([128, LQC], F32R, tag="expS")
                            split_exp = False
                            for (a, b2) in segs:
                                nc.tensor.matmul(
                                    sc_ps[:, a:b2],
                                    kT_sb[base:base + 64, 128 * t:128 * t + 128],
                                    qT_sb[base:base + 64, mt, LQC * c + a:LQC * c + b2],
                                    start=True, stop=True)
                                if 128 * t >= LQC * c and a <= o < b2:
                                    # triangular -1e32 mask on PE (bf16, 53ns)
                                    # so scores->exp never crosses DVE
                                    nc.tensor.matmul(sc_ps[:, o:o + 128], triT_sb,
                                                     idnb_sb, start=False,
                                                     stop=True)
                                if split_exp:
                                    nc.scalar.activation(out=expS[:, a:b2],
                                                         in_=sc_ps[:, a:b2],
                                                         func=EXP, scale=0.125)
                            if not split_exp:
                                nc.scalar.activation(out=expS[:, o:LQC],
                                                     in_=sc_ps[:, o:LQC],
                                                     func=EXP, scale=0.125)
                        else:
                            expS = work.tile([128, LQC], F32R, tag="expS")
                            for (a, b2) in segs:
                                w = b2 - a
                                sc_ps = scps.tile([128, 512], F32, tag="sc")
                                nc.tensor.matmul(
                                    sc_ps[:, 0:w],
                                    kT_sb[base:base + 64, 128 * t:128 * t + 128],
                                    qT_sb[base:base + 64, mt, LQC * c + a:LQC * c + b2],
                                    start=True, stop=True)
                                if 128 * t >= LQC * c and a <= o < b2:
                                    nc.tensor.matmul(sc_ps[:, o - a:o - a + 128],
                                                     triT_sb, idnb_sb,
                                                     start=False, stop=True)
                                nc.scalar.activation(out=expS[:, a:b2],
                                                     in_=sc_ps[:, 0:w],
                                                     func=EXP, scale=0.125)
                        if pending_pv is not None:
                            pending_pv()
                        pv_now, seg_now = pv_ps, segs
                        st_now, sp_now = ti == 0, ti == ntile - 1
                        ex_now, j_now, t_now = expS, j, t

                        def emit_pv(pv_t=pv_now, segs_t=seg_now, st=st_now,
                                    sp=sp_now, ex=ex_now, jj=j_now, tt=t_now):
                            for (a, b2) in segs_t:
                                nc.tensor.matmul(pv_t[:, a:b2],
                                                 v_aug[:, tt, 65 * jj:65 * jj + 65],
                                                 ex[:, a:b2],
                                                 start=st, stop=sp)
                        pending_pv = emit_pv
                    return pv_ps

                def outproj_group(ych, ot, pool, tag, od):
                    o_ps = pool.tile([128, 1024], F32, tag=tag)
                    for half in range(2):
                        yoff = 512 * half
                        for it in range(4):
                            nc.tensor.matmul(
                                o_ps[:, yoff:yoff + 512],
                                ow_sb[:, it, 128 * ot:128 * ot + 128],
                                yT_cs[ych][:, it, yoff:yoff + 512],
                                start=(it == 0), stop=(it == 3))
                    o_sb = od.tile([128, 1024], F32, tag="osb")
                    nc.vector.tensor_copy(out=o_sb, in_=o_ps)
                    nc.sync.dma_start(
                        out=outp.ap()[ot, :, 1024 * ych:1024 * ych + 1024],
                        in_=o_sb)

                pending_pv = None
                # within c1, do h>=4 first: their y lands via SBUF->SBUF
                # DMA (+900ns sem), so the final tails before out_proj are
                # the cheap direct DVE writes
                jobs = [(0, h) for h in range(8)] + \
                       [(1, h) for h in [4, 5, 6, 7, 0, 1, 2, 3]]
                with tc.tile_pool(name="od", bufs=3) as od:
                    with tc.tile_pool(name="scps", bufs=2, space="PSUM") as scps, \
                         tc.tile_pool(name="pvps", bufs=2, space="PSUM") as pvps:
                        pending = None
                        for (c, h) in jobs:
                            pv_ps = emit_head(c, h, scps, pvps, wide=True)
                            if pending is not None:
                                head_tail(*pending)
                            pending = (c, h, pv_ps)
                        pending_pv()
                        pending_pv = None
                        head_tail(*pending)
                    # ---- Phase D ----
                    with tc.tile_pool(name="ops", bufs=2, space="PSUM") as ops:
                        for ych in range(2):
                            for ot in range(NKT):
                                outproj_group(ych, ot, ops, "o", od)
            owp.__exit__(None, None, None)
    nc.compile()
    return nc


def _perm512():
    p = np.empty(512, dtype=np.int64)
    for mt in range(4):
        for half in range(2):
            head = mt + 4 * half
            p[128 * mt + 64 * half:128 * mt + 64 * half + 64] = \
                np.arange(64 * head, 64 * head + 64)
    return p


def kernel(x, attention_mask, q_w, q_b, k_w, k_b, v_w, v_b, o_w, o_b):
    from concourse.bass_utils import run_bass_kernel_spmd

    x = np.asarray(x, dtype=np.float32)
    q_w = np.asarray(q_w, dtype=np.float32); q_b = np.asarray(q_b, dtype=np.float32)
    k_w = np.asarray(k_w, dtype=np.float32); k_b = np.asarray(k_b, dtype=np.float32)
    v_w = np.asarray(v_w, dtype=np.float32); v_b = np.asarray(v_b, dtype=np.float32)
    o_w = np.asarray(o_w, dtype=np.float32); o_b = np.asarray(o_b, dtype=np.float32)
    am = np.asarray(attention_mask)
    assert am.all(), "kernel assumes attention_mask == all ones"

    if "nc" not in _cached:
        _cached["nc"] = _build()
    nc = _cached["nc"]

    import ml_dtypes
    perm = _perm512()
    tri_np = np.where(np.arange(128)[:, None] > np.arange(128)[None, :],
                      np.float32(BIG), np.float32(0)).astype(np.float32)
    triT_np = tri_np.T.astype(ml_dtypes.bfloat16)
    idb_np = np.eye(128, dtype=ml_dtypes.bfloat16)
    id_np = np.eye(128, dtype=np.float32)

    in_maps = []
    for c in range(NCORE):
        b, g = c // 4, c % 4
        G0 = 512 * g
        xT_t = np.ascontiguousarray(
            x[b].T.reshape(NKT, 128, L).transpose(1, 0, 2))
        qws = q_w[G0:G0 + 512][perm]
        qw_t = np.ascontiguousarray(qws.T.reshape(NKT, 128, 512).transpose(1, 0, 2))
        kws = k_w[128 * g:128 * g + 128]
        kw_t = np.ascontiguousarray(kws.T.reshape(NKT, 128, 128).transpose(1, 0, 2))
        vws = v_w[128 * g:128 * g + 128]
        vw_t = np.ascontiguousarray(vws.T.reshape(NKT, 128, 128).transpose(1, 0, 2))
        owp = o_w[:, G0:G0 + 512][:, perm]
        ow_t = np.ascontiguousarray(owp.T.reshape(4, 128, HID).transpose(1, 0, 2))
        qb_t = np.ascontiguousarray(q_b[G0:G0 + 512][perm].reshape(4, 128).T)
        kb_t = k_b[128 * g:128 * g + 128].reshape(128, 1).copy()
        vb_t = v_b[128 * g:128 * g + 128].reshape(128, 1).copy()
        in_maps.append({"xT": xT_t, "qw": qw_t, "kw": kw_t, "vw": vw_t,
                        "ow": ow_t, "qb": qb_t, "kb": kb_t, "vb": vb_t,
                        "triT": triT_np, "idnb": idb_np, "ident": id_np})

    res = run_bass_kernel_spmd(nc, in_maps, core_ids=list(range(NCORE)))
    out = np.empty((2, L, HID), dtype=np.float32)
    for b in range(2):
        acc = res.results[4 * b]["outp"].astype(np.float32).copy()
        for i in range(1, 4):
            acc += res.results[4 * b + i]["outp"]
        out[b] = acc.reshape(HID, L).T + o_b
    return out

